# revision 11
# baseline (speedup 1.0000x reference)
"""Leave-one-out logsumexp kernel for Trainium2 (8 NeuronCores, SPMD).

Problem: logits [131072, 1000] f32 ->
    out[b, k] = -logsumexp(logits[b, :] without column k)

Math (per row):
    s   = sum_j exp(x_j)            (no max subtraction needed: |x| <~ 6
                                     for standard-normal inputs, exp fits
                                     comfortably in fp32)
    out_k = -ln(s - exp(x_k))

Per-core pipeline (batch sharded 8 ways, 16384 rows/core):
    tile = 128 partitions x (M=4 rows/partition) x 1000 cols  (2 MB DMAs)
    ACT:  e = Exp(x)        with accum_out -> s  (free running sum)
    ACT:  l = Ln(-1*e + s)  (scale=-1, per-partition bias=s)
    DVE:  out = -l
This is DMA-bound: 65.5 MB in + 65.5 MB out per core @ ~358 GB/s.
"""

from contextlib import ExitStack

import numpy as np

import concourse.tile as tile
from concourse import bacc, mybir
from concourse.bass_utils import run_bass_kernel_spmd

N_CORES = 8
B, K = 131072, 1000
BS = B // N_CORES  # 16384 rows per core
P = 128            # SBUF partitions
M = 8              # rows per partition per tile (4 MB DMAs)
BUFS = 5
INPLACE = True

_nc_cache = {}


class _Bacc(bacc.Bacc):
    """Bacc that pins the ACT table set to natural_log_exp_and_others.

    The default per-activation greedy choice alternates exp_and_others /
    natural_log per tile -> 64 LoadActFuncSet x ~2.7us of pure ACT stall.
    Both Exp and Ln live in one set; blanking every other set's function
    list (indices preserved - the id is the list position) makes the
    fixpoint pass emit exactly one load.
    """

    def insert_act_table_loads(self):
        import bass_rust as _bass_rust
        from concourse.hw_specs import get_activation_tables
        from concourse import mybir as _mb

        has_activation = any(
            isinstance(i, _mb.InstActivation)
            for b in self.main_func.blocks
            for i in b.instructions
        )
        if not has_activation:
            return
        keep = "natural_log_exp_and_others"
        tables = [
            (name, funcs if name == keep else set())
            for name, funcs in get_activation_tables(self.m.arch).items()
        ]
        _bass_rust.insert_act_table_loads(self, tables)


def _build_nc(reps: int = 1, m: int = M, bufs: int = BUFS, inplace: bool = INPLACE):
    """Build the SPMD kernel. reps>1 repeats the whole body inside one
    NEFF (same in/out, idempotent) — used only for timing calibration."""
    nc = _Bacc()
    f32 = mybir.dt.float32
    x = nc.declare_dram_parameter("x", [BS, K], f32, isOutput=False)
    out = nc.declare_dram_parameter("out", [BS, K], f32, isOutput=True)

    rows_per_tile = P * m
    n_tiles = BS // rows_per_tile
    free = m * K

    # tile t, partition p holds rows t*rows + p*m + {0..m-1}, contiguous
    xr = x.rearrange("(t p m) k -> t p (m k)", p=P, m=m)
    outr = out.rearrange("(t p m) k -> t p (m k)", p=P, m=m)

    with tile.TileContext(nc) as tc, ExitStack() as ctx:
        xpool = ctx.enter_context(tc.tile_pool(name="x", bufs=bufs))
        spool = ctx.enter_context(tc.tile_pool(name="s", bufs=bufs))
        ypool = (
            xpool
            if inplace
            else ctx.enter_context(tc.tile_pool(name="y", bufs=bufs))
        )

        for _ in range(reps):
            for t in range(n_tiles):
                xt = xpool.tile([P, free], f32)
                nc.sync.dma_start(out=xt[:], in_=xr[t])

                st = spool.tile([P, m], f32)
                yt = xt if inplace else ypool.tile([P, free], f32)
                for j in range(m):
                    sl = slice(j * K, (j + 1) * K)
                    nc.scalar.activation(
                        out=yt[:, sl],
                        in_=xt[:, sl],
                        func=mybir.ActivationFunctionType.Exp,
                        accum_out=st[:, j : j + 1],
                    )
                for j in range(m):
                    sl = slice(j * K, (j + 1) * K)
                    nc.scalar.activation(
                        out=xt[:, sl],
                        in_=yt[:, sl],
                        func=mybir.ActivationFunctionType.Ln,
                        bias=st[:, j : j + 1],
                        scale=-1.0,
                    )
                nc.vector.tensor_scalar_mul(yt[:], xt[:], -1.0)
                nc.sync.dma_start(out=outr[t], in_=yt[:])
    nc.compile()
    return nc


def kernel(logits: np.ndarray) -> np.ndarray:
    assert logits.shape == (B, K), logits.shape
    logits = np.ascontiguousarray(logits, dtype=np.float32)

    if "nc" not in _nc_cache:
        _nc_cache["nc"] = _build_nc()
    nc = _nc_cache["nc"]

    in_maps = [
        {"x": logits[i * BS : (i + 1) * BS]} for i in range(N_CORES)
    ]
    res = run_bass_kernel_spmd(nc, in_maps, list(range(N_CORES)))
    return np.concatenate(
        [res.results[i]["out"] for i in range(N_CORES)], axis=0
    )


# revision 12
# speedup vs baseline: 1.0703x; 1.0703x over previous
"""Leave-one-out logsumexp kernel for Trainium2 (8 NeuronCores, SPMD).

Problem: logits [131072, 1000] f32 ->
    out[b, k] = -logsumexp(logits[b, :] without column k)

Math (per row):
    s   = sum_j exp(x_j)            (no max subtraction needed: |x| <~ 6
                                     for standard-normal inputs, exp fits
                                     comfortably in fp32)
    out_k = -ln(s - exp(x_k))

Per-core pipeline (batch sharded 8 ways, 16384 rows/core):
    tile = 128 partitions x (M=4 rows/partition) x 1000 cols  (2 MB DMAs)
    ACT:  e = Exp(x)        with accum_out -> s  (free running sum)
    ACT:  l = Ln(-1*e + s)  (scale=-1, per-partition bias=s)
    DVE:  out = -l
This is DMA-bound: 65.5 MB in + 65.5 MB out per core @ ~358 GB/s.
"""

from contextlib import ExitStack

import numpy as np

import concourse.tile as tile
from concourse import bacc, mybir
from concourse.bass_utils import run_bass_kernel_spmd

N_CORES = 8
B, K = 131072, 1000
BS = B // N_CORES  # 16384 rows per core
P = 128            # SBUF partitions
M = 8              # rows per partition per tile (4 MB DMAs)
BUFS = 5
INPLACE = True

_nc_cache = {}


class _Bacc(bacc.Bacc):
    """Bacc that pins the ACT table set to natural_log_exp_and_others.

    The default per-activation greedy choice alternates exp_and_others /
    natural_log per tile -> 64 LoadActFuncSet x ~2.7us of pure ACT stall.
    Both Exp and Ln live in one set; blanking every other set's function
    list (indices preserved - the id is the list position) makes the
    fixpoint pass emit exactly one load.
    """

    def insert_act_table_loads(self):
        import bass_rust as _bass_rust
        from concourse.hw_specs import get_activation_tables
        from concourse import mybir as _mb

        has_activation = any(
            isinstance(i, _mb.InstActivation)
            for b in self.main_func.blocks
            for i in b.instructions
        )
        if not has_activation:
            return
        keep = "natural_log_exp_and_others"
        all_tables = get_activation_tables(self.m.arch)
        if keep not in all_tables:
            return super().insert_act_table_loads()
        tables = [
            (name, funcs if name == keep else set())
            for name, funcs in all_tables.items()
        ]
        _bass_rust.insert_act_table_loads(self, tables)


def _build_nc(reps: int = 1, m: int = M, bufs: int = BUFS, inplace: bool = INPLACE):
    """Build the SPMD kernel. reps>1 repeats the whole body inside one
    NEFF (same in/out, idempotent) — used only for timing calibration."""
    nc = _Bacc()
    f32 = mybir.dt.float32
    x = nc.declare_dram_parameter("x", [BS, K], f32, isOutput=False)
    out = nc.declare_dram_parameter("out", [BS, K], f32, isOutput=True)

    rows_per_tile = P * m
    n_tiles = BS // rows_per_tile
    free = m * K

    # tile t, partition p holds rows t*rows + p*m + {0..m-1}, contiguous
    xr = x.rearrange("(t p m) k -> t p (m k)", p=P, m=m)
    outr = out.rearrange("(t p m) k -> t p (m k)", p=P, m=m)

    with tile.TileContext(nc) as tc, ExitStack() as ctx:
        xpool = ctx.enter_context(tc.tile_pool(name="x", bufs=bufs))
        spool = ctx.enter_context(tc.tile_pool(name="s", bufs=bufs))
        ypool = (
            xpool
            if inplace
            else ctx.enter_context(tc.tile_pool(name="y", bufs=bufs))
        )

        for _ in range(reps):
            for t in range(n_tiles):
                xt = xpool.tile([P, free], f32)
                nc.sync.dma_start(out=xt[:], in_=xr[t])

                st = spool.tile([P, m], f32)
                yt = xt if inplace else ypool.tile([P, free], f32)
                for j in range(m):
                    sl = slice(j * K, (j + 1) * K)
                    nc.scalar.activation(
                        out=yt[:, sl],
                        in_=xt[:, sl],
                        func=mybir.ActivationFunctionType.Exp,
                        accum_out=st[:, j : j + 1],
                    )
                for j in range(m):
                    sl = slice(j * K, (j + 1) * K)
                    nc.scalar.activation(
                        out=xt[:, sl],
                        in_=yt[:, sl],
                        func=mybir.ActivationFunctionType.Ln,
                        bias=st[:, j : j + 1],
                        scale=-1.0,
                    )
                nc.vector.tensor_scalar_mul(yt[:], xt[:], -1.0)
                nc.sync.dma_start(out=outr[t], in_=yt[:])
    nc.compile()
    return nc


def kernel(logits: np.ndarray) -> np.ndarray:
    assert logits.shape == (B, K), logits.shape
    logits = np.ascontiguousarray(logits, dtype=np.float32)

    if "nc" not in _nc_cache:
        _nc_cache["nc"] = _build_nc()
    nc = _nc_cache["nc"]

    in_maps = [
        {"x": logits[i * BS : (i + 1) * BS]} for i in range(N_CORES)
    ]
    res = run_bass_kernel_spmd(nc, in_maps, list(range(N_CORES)))
    return np.concatenate(
        [res.results[i]["out"] for i in range(N_CORES)], axis=0
    )
